# revision 1
# baseline (speedup 1.0000x reference)
"""DiT 2-block forward on 8 trn2 NeuronCores.

Strategy: sequence-parallel matmuls (each core owns 384 tokens, full weights,
channel-major activations) + head-parallel attention core (2 heads/core),
with 4 AllToAll exchanges per block. All matmuls bf16 x bf16 -> fp32 PSUM;
residual stream fp32 in SBUF. AdaLN modulation vectors precomputed on host.
"""
import numpy as np
import ml_dtypes

import concourse.bass as bass
import concourse.mybir as mybir
import concourse.tile as tile
from concourse import bacc
from concourse.bass_utils import run_bass_kernel_spmd

P = 128
L, D, H, HD, S, NB, DFF = 3072, 1024, 16, 64, 512, 2, 4096
NC = 8
LC = L // NC            # 384 tokens per core
DCH = D // P            # 8 din chunks
LCH = L // 512          # 6 l-chunks
MT = L // P             # 24 m-tiles (self)
MTC = S // P            # 4 m-tiles (cross)
FCH = DFF // P          # 32 dff chunks
NV = 9                  # per-block channel vectors
dt = mybir.dt
AF = mybir.ActivationFunctionType
ALU = mybir.AluOpType
BF = ml_dtypes.bfloat16

_cache = {}


def _build():
    nc = bacc.Bacc("TRN2", target_bir_lowering=False, debug=False,
                   enable_asserts=True, num_devices=NC)

    # ---------------- inputs ----------------
    x_t = nc.dram_tensor("x_t", [P, DCH * LC], dt.float32, kind="ExternalInput")
    ctx_t = nc.dram_tensor("ctx_t", [P, DCH * S], dt.bfloat16, kind="ExternalInput")
    cos2 = nc.dram_tensor("cos2", [P, L], dt.bfloat16, kind="ExternalInput")
    ss2 = nc.dram_tensor("ss2", [P, L], dt.bfloat16, kind="ExternalInput")
    permT = nc.dram_tensor("permT", [P, P], dt.bfloat16, kind="ExternalInput")
    vecs = nc.dram_tensor("vecs", [P, NB * NV * DCH], dt.float32, kind="ExternalInput")
    f1bv = nc.dram_tensor("f1bv", [P, NB * FCH], dt.float32, kind="ExternalInput")
    wqk = nc.dram_tensor("wqk", [NB, 16, P, DCH, P], dt.bfloat16, kind="ExternalInput")
    wv = nc.dram_tensor("wv", [NB, 2, P, DCH, 512], dt.bfloat16, kind="ExternalInput")
    wso = nc.dram_tensor("wso", [NB, DCH, P, DCH, P], dt.bfloat16, kind="ExternalInput")
    wcq = nc.dram_tensor("wcq", [NB, DCH, P, DCH, P], dt.bfloat16, kind="ExternalInput")
    wck = nc.dram_tensor("wck", [NB, P, DCH, P], dt.bfloat16, kind="ExternalInput")
    wcv = nc.dram_tensor("wcv", [NB, P, DCH, P], dt.bfloat16, kind="ExternalInput")
    wco = nc.dram_tensor("wco", [NB, DCH, P, DCH, P], dt.bfloat16, kind="ExternalInput")
    wf1 = nc.dram_tensor("wf1", [NB, FCH, P, DCH, P], dt.bfloat16, kind="ExternalInput")
    wf2 = nc.dram_tensor("wf2", [NB, DCH, 4, P, DCH, P], dt.bfloat16, kind="ExternalInput")
    out_t = nc.dram_tensor("out_t", [P, DCH * LC], dt.float32, kind="ExternalOutput")

    RG = [list(range(NC))]

    from contextlib import ExitStack
    with tile.TileContext(nc) as tc, ExitStack() as ctx:
        cpool = ctx.enter_context(tc.tile_pool(name="cpool", bufs=1))
        spool = ctx.enter_context(tc.tile_pool(name="spool", bufs=1))
        wpool = ctx.enter_context(tc.tile_pool(name="wpool", bufs=6))
        wvpool = ctx.enter_context(tc.tile_pool(name="wvpool", bufs=1))
        ppool = ctx.enter_context(tc.tile_pool(name="ppool", bufs=3))
        vpool = ctx.enter_context(tc.tile_pool(name="vpool", bufs=25))
        vcpool = ctx.enter_context(tc.tile_pool(name="vcpool", bufs=6))
        opool = ctx.enter_context(tc.tile_pool(name="opool", bufs=4))
        bigp = ctx.enter_context(tc.tile_pool(name="bigp", bufs=1))
        stg = ctx.enter_context(tc.tile_pool(name="stg", bufs=2))
        psA = ctx.enter_context(tc.tile_pool(name="psA", bufs=2, space="PSUM"))
        psB = ctx.enter_context(tc.tile_pool(name="psB", bufs=4, space="PSUM"))
        dram = ctx.enter_context(tc.tile_pool(name="dram", bufs=1, space="DRAM"))

        # ---------- persistent tiles ----------
        x_sb = cpool.tile([P, DCH, LC], dt.float32, tag="x_sb")
        nc.sync.dma_start(x_sb[:], x_t.ap().rearrange("p (o t) -> p o t", t=LC))
        ctxb = cpool.tile([P, DCH, S], dt.bfloat16, tag="ctxb")
        nc.sync.dma_start(ctxb[:], ctx_t.ap().rearrange("p (o t) -> p o t", t=S))
        cos_sb = cpool.tile([P, L], dt.bfloat16, tag="cos_sb")
        nc.sync.dma_start(cos_sb[:], cos2.ap())
        ss_sb = cpool.tile([P, L], dt.bfloat16, tag="ss_sb")
        nc.sync.dma_start(ss_sb[:], ss2.ap())
        pT_sb = cpool.tile([P, P], dt.bfloat16, tag="pT_sb")
        nc.sync.dma_start(pT_sb[:], permT.ap())
        vec_sb = cpool.tile([P, NB * NV * DCH], dt.float32, tag="vec_sb")
        nc.sync.dma_start(vec_sb[:], vecs.ap())
        f1b_sb = cpool.tile([P, NB * FCH], dt.float32, tag="f1b_sb")
        nc.sync.dma_start(f1b_sb[:], f1bv.ap())
        ones1 = cpool.tile([P, 1], dt.bfloat16, tag="ones1")
        nc.gpsimd.memset(ones1[:], 1.0)
        epsb = cpool.tile([P, 1], dt.float32, tag="epsb")
        nc.gpsimd.memset(epsb[:], 1e-6)

        def vap(i, v, j=None):
            base = (i * NV + v) * DCH
            if j is None:
                return vec_sb[:, base:base + DCH]
            return vec_sb[:, base + j:base + j + 1]

        # ---------- layernorm ----------
        def emit_ln(i, vmod, out_hx):
            """out_hx [P, DCH, LC] bf16 = ln(x)*(1+sc)+sh (vmod=(sc1_idx, sh_idx)) or ln(x)."""
            xb = spool.tile([P, DCH, LC], dt.bfloat16, tag="xb")
            nc.vector.tensor_copy(xb[:], x_sb[:])
            ps1 = psB.tile([P, 512], dt.float32, tag="psb")
            for o in range(DCH):
                nc.tensor.matmul(ps1[:1, :LC], ones1[:], xb[:, o, :],
                                 start=(o == 0), stop=(o == DCH - 1))
            nc.scalar.activation(xb[:], xb[:], AF.Square)  # in-place square
            ps2 = psB.tile([P, 512], dt.float32, tag="psb")
            for o in range(DCH):
                nc.tensor.matmul(ps2[:1, :LC], ones1[:], xb[:, o, :],
                                 start=(o == 0), stop=(o == DCH - 1))
            mrow = spool.tile([1, LC], dt.float32, tag="mrow")
            nc.vector.tensor_scalar_mul(mrow[:], ps1[:1, :LC], 1.0 / D)
            msq = spool.tile([1, LC], dt.float32, tag="msq")
            nc.scalar.activation(msq[:], mrow[:], AF.Square)
            varr = spool.tile([1, LC], dt.float32, tag="varr")
            nc.vector.tensor_scalar(varr[:], ps2[:1, :LC], 1.0 / D, None, ALU.mult)
            nc.vector.tensor_tensor(varr[:], varr[:], msq[:], ALU.subtract)
            rs = spool.tile([1, LC], dt.float32, tag="rs")
            nc.scalar.activation(rs[:], varr[:], AF.Abs_reciprocal_sqrt, bias=epsb[:1])
            mrs = spool.tile([1, LC], dt.float32, tag="mrs")
            nc.vector.tensor_tensor(mrs[:], mrow[:], rs[:], ALU.mult)
            rsb = spool.tile([P, LC], dt.float32, tag="rsb")
            nc.gpsimd.partition_broadcast(rsb[:], rs[:1])
            mrsb = spool.tile([P, LC], dt.float32, tag="mrsb")
            nc.gpsimd.partition_broadcast(mrsb[:], mrs[:1])
            t = spool.tile([P, DCH, LC], dt.float32, tag="lnt")
            nc.vector.tensor_tensor(t[:], x_sb[:],
                                    rsb[:, None, :].to_broadcast([P, DCH, LC]), ALU.mult)
            nc.vector.tensor_tensor(t[:], t[:],
                                    mrsb[:, None, :].to_broadcast([P, DCH, LC]), ALU.subtract)
            if vmod is not None:
                sc1v, shv = vmod
                nc.vector.tensor_tensor(t[:], t[:],
                                        vap(i, sc1v)[:, :, None].to_broadcast([P, DCH, LC]),
                                        ALU.mult)
                nc.vector.tensor_tensor(out_hx[:], t[:],
                                        vap(i, shv)[:, :, None].to_broadcast([P, DCH, LC]),
                                        ALU.add)
            else:
                nc.vector.tensor_copy(out_hx[:], t[:])

        # ---------- blocks ----------
        for i in range(NB):
            # ===== cross K/V (independent of x; fills PE early) =====
            k2c = bigp.tile([P, S], dt.bfloat16, tag="k2c")
            wt = wpool.tile([P, DCH, P], dt.bfloat16, tag="w8")
            nc.sync.dma_start(wt[:], wck.ap()[i])
            psk = psB.tile([P, 512], dt.float32, tag="psb")
            for o in range(DCH):
                nc.tensor.matmul(psk[:, :S], wt[:, o, :], ctxb[:, o, :],
                                 start=(o == 0), stop=(o == DCH - 1))
            nc.vector.tensor_copy(k2c[:], psk[:, :S])
            wtv = wpool.tile([P, DCH, P], dt.bfloat16, tag="w8")
            nc.sync.dma_start(wtv[:], wcv.ap()[i])
            vextc = []
            for tcix in range(MTC):
                vt = vcpool.tile([P, 130], dt.bfloat16, tag="vextc")
                nc.gpsimd.memset(vt[:], 1.0)
                psv = psB.tile([P, 512], dt.float32, tag="psb")
                for o in range(DCH):
                    nc.tensor.matmul(psv[:, :P], ctxb[:, o, tcix * P:(tcix + 1) * P],
                                     wtv[:, o, :], start=(o == 0), stop=(o == DCH - 1))
                nc.vector.tensor_copy(vt[:, 0:64], psv[:, 0:64])
                nc.vector.tensor_copy(vt[:, 65:129], psv[:, 64:128])
                vextc.append(vt)

            # ===== self-attention =====
            hx = spool.tile([P, DCH, LC], dt.bfloat16, tag="hx")
            emit_ln(i, (1, 0), hx)

            qkv_in = dram.tile([NC, 3 * P * LC], dt.bfloat16, tag="qkv_in")
            qkv_out = dram.tile([NC, 3 * P * LC], dt.bfloat16, tag="qkv_out")
            # q, k projections (channel-major lhsT tiles)
            for j in range(16):
                wt = wpool.tile([P, DCH, P], dt.bfloat16, tag="w8")
                nc.sync.dma_start(wt[:], wqk.ap()[i, j])
                pp = psB.tile([P, 512], dt.float32, tag="psb")
                for o in range(DCH):
                    nc.tensor.matmul(pp[:, :LC], wt[:, o, :], hx[:, o, :],
                                     start=(o == 0), stop=(o == DCH - 1))
                st = stg.tile([P, LC], dt.bfloat16, tag="stg384")
                nc.vector.tensor_copy(st[:], pp[:, :LC])
                region = 0 if j < 8 else 1
                shard = j % 8
                dst = qkv_in[shard, region * P * LC:(region + 1) * P * LC] \
                    .rearrange("(a b) -> a b", b=LC)
                nc.sync.dma_start(dst, st[:])
            # v projection (token-major)
            for g in range(2):
                wvt = wvpool.tile([P, DCH, 512], dt.bfloat16, tag="wv")
                nc.sync.dma_start(wvt[:], wv.ap()[i, g])
                for tcix in range(LC // P):
                    pp = psB.tile([P, 512], dt.float32, tag="psb")
                    for o in range(DCH):
                        nc.tensor.matmul(pp[:], hx[:, o, tcix * P:(tcix + 1) * P],
                                         wvt[:, o, :], start=(o == 0), stop=(o == DCH - 1))
                    st = stg.tile([P, 512], dt.bfloat16, tag="stg512")
                    nc.vector.tensor_copy(st[:], pp[:])
                    for s4 in range(4):
                        shard = 4 * g + s4
                        dstv = qkv_in[shard, 2 * P * LC:3 * P * LC] \
                            .rearrange("(t c) -> t c", c=P)[tcix * P:(tcix + 1) * P, :]
                        nc.sync.dma_start(dstv, st[:, s4 * P:(s4 + 1) * P])

            nc.gpsimd.collective_compute("AllToAll", ALU.bypass, replica_groups=RG,
                                         ins=[qkv_in.opt()], outs=[qkv_out.opt()])

            q_sb = bigp.tile([P, L], dt.bfloat16, tag="q_sb")
            k_sb = bigp.tile([P, L], dt.bfloat16, tag="k_sb")
            for p in range(NC):
                nc.sync.dma_start(
                    q_sb[:, p * LC:(p + 1) * LC],
                    qkv_out[p, 0:P * LC].rearrange("(a b) -> a b", b=LC))
                nc.sync.dma_start(
                    k_sb[:, p * LC:(p + 1) * LC],
                    qkv_out[p, P * LC:2 * P * LC].rearrange("(a b) -> a b", b=LC))
            vext = []
            for p in range(NC):
                vreg = qkv_out[p, 2 * P * LC:3 * P * LC].rearrange("(t c) -> t c", c=P)
                for tcl in range(LC // P):
                    vt = vpool.tile([P, 130], dt.bfloat16, tag="vext")
                    nc.gpsimd.memset(vt[:], 1.0)
                    nc.sync.dma_start(vt[:, 0:64], vreg[tcl * P:(tcl + 1) * P, 0:64])
                    nc.sync.dma_start(vt[:, 65:129], vreg[tcl * P:(tcl + 1) * P, 64:128])
                    vext.append(vt)

            # RoPE (perm matmul + combine)
            qr = bigp.tile([P, L], dt.bfloat16, tag="qr")
            kr = bigp.tile([P, L], dt.bfloat16, tag="kr")
            for src, dstt in ((q_sb, qr), (k_sb, kr)):
                for lc in range(LCH):
                    sl = slice(lc * 512, (lc + 1) * 512)
                    psr = psB.tile([P, 512], dt.float32, tag="psb")
                    nc.tensor.matmul(psr[:], pT_sb[:], src[:, sl], start=True, stop=True)
                    nc.vector.tensor_tensor(dstt[:, sl], src[:, sl], cos_sb[:, sl], ALU.mult)
                    rt = stg.tile([P, 512], dt.bfloat16, tag="rtmp")
                    nc.vector.tensor_tensor(rt[:], psr[:], ss_sb[:, sl], ALU.mult)
                    nc.vector.tensor_tensor(dstt[:, sl], dstt[:, sl], rt[:], ALU.add)

            # flash attention (l-chunk outer, m inner); M=65 lhsT carries ones col
            o_h0 = opool.tile([65, L], dt.bfloat16, tag="osb")
            o_h1 = opool.tile([65, L], dt.bfloat16, tag="osb")
            for lc in range(LCH):
                sl = slice(lc * 512, (lc + 1) * 512)
                pso0 = psB.tile([P, 512], dt.float32, tag="psb")
                pso1 = psB.tile([P, 512], dt.float32, tag="psb")
                for mt in range(MT):
                    pqk = psA.tile([P, 1024], dt.float32, tag="psa")
                    nc.tensor.matmul(pqk[:, 0:512], kr[0:64, mt * P:(mt + 1) * P],
                                     qr[0:64, sl], start=True, stop=True)
                    nc.tensor.matmul(pqk[:, 512:1024], kr[64:128, mt * P:(mt + 1) * P],
                                     qr[64:128, sl], start=True, stop=True)
                    Pt = ppool.tile([P, 1024], dt.bfloat16, tag="Pt")
                    nc.scalar.activation(Pt[:], pqk[:], AF.Exp, scale=HD ** -0.5)
                    nc.tensor.matmul(pso0[:65, :], vext[mt][:, 0:65], Pt[:, 0:512],
                                     start=(mt == 0), stop=(mt == MT - 1))
                    nc.tensor.matmul(pso1[:65, :], vext[mt][:, 65:130], Pt[:, 512:1024],
                                     start=(mt == 0), stop=(mt == MT - 1))
                nc.vector.tensor_copy(o_h0[:, sl], pso0[:65, :])
                nc.vector.tensor_copy(o_h1[:, sl], pso1[:65, :])

            # ===== o exchange + o-proj + residual =====
            oa_in = dram.tile([NC, 2, 65, LC], dt.bfloat16, tag="oa_in")
            oa_out = dram.tile([NC, 2, 65, LC], dt.bfloat16, tag="oa_out")
            for p in range(NC):
                nc.sync.dma_start(oa_in[p, 0], o_h0[:, p * LC:(p + 1) * LC])
                nc.sync.dma_start(oa_in[p, 1], o_h1[:, p * LC:(p + 1) * LC])
            nc.gpsimd.collective_compute("AllToAll", ALU.bypass, replica_groups=RG,
                                         ins=[oa_in.opt()], outs=[oa_out.opt()])

            def recv_o(oa_out_b, tagp):
                orecv = spool.tile([P, DCH, LC], dt.bfloat16, tag="orecv")
                for p in range(NC):
                    nc.sync.dma_start(orecv[0:64, p, :], oa_out_b[p, 0, 0:64, :])
                    nc.sync.dma_start(orecv[64:128, p, :], oa_out_b[p, 1, 0:64, :])
                sums = spool.tile([16, LC], dt.bfloat16, tag="sums")
                nc.sync.dma_start(sums[:], oa_out_b[:, :, 64, :])
                rcp = spool.tile([16, LC], dt.float32, tag="rcp")
                nc.vector.reciprocal(rcp[:], sums[:])
                rdram = dram.tile([16, LC], dt.float32, tag=f"rdram{tagp}")
                nc.sync.dma_start(rdram[:], rcp[:])
                rbq = spool.tile([P, DCH, LC], dt.float32, tag="rbq")
                rv = rdram[:].rearrange("(o two) t -> two o t", two=2)
                nc.sync.dma_start(rbq[0:64, :, :], rv[0].partition_broadcast(64))
                nc.sync.dma_start(rbq[64:128, :, :], rv[1].partition_broadcast(64))
                nc.vector.tensor_tensor(orecv[:], orecv[:], rbq[:], ALU.mult)
                return orecv

            onorm = recv_o(oa_out, "s")
            for j in range(DCH):
                wt = wpool.tile([P, DCH, P], dt.bfloat16, tag="w8")
                nc.sync.dma_start(wt[:], wso.ap()[i, j])
                pp = psB.tile([P, 512], dt.float32, tag="psb")
                for o in range(DCH):
                    nc.tensor.matmul(pp[:, :LC], wt[:, o, :], onorm[:, o, :],
                                     start=(o == 0), stop=(o == DCH - 1))
                tmp = stg.tile([P, LC], dt.float32, tag="resid")
                nc.vector.tensor_scalar(tmp[:], pp[:, :LC], vap(i, 2, j), vap(i, 3, j),
                                        ALU.mult, ALU.add)
                nc.vector.tensor_tensor(x_sb[:, j, :], x_sb[:, j, :], tmp[:], ALU.add)

            # ===== cross-attention =====
            hx2 = spool.tile([P, DCH, LC], dt.bfloat16, tag="hx")
            emit_ln(i, None, hx2)
            cq_in = dram.tile([NC, P, LC], dt.bfloat16, tag="cq_in")
            cq_out = dram.tile([NC, P, LC], dt.bfloat16, tag="cq_out")
            for j in range(DCH):
                wt = wpool.tile([P, DCH, P], dt.bfloat16, tag="w8")
                nc.sync.dma_start(wt[:], wcq.ap()[i, j])
                pp = psB.tile([P, 512], dt.float32, tag="psb")
                for o in range(DCH):
                    nc.tensor.matmul(pp[:, :LC], wt[:, o, :], hx2[:, o, :],
                                     start=(o == 0), stop=(o == DCH - 1))
                st = stg.tile([P, LC], dt.bfloat16, tag="stg384")
                nc.vector.tensor_copy(st[:], pp[:, :LC])
                nc.sync.dma_start(cq_in[j], st[:])
            nc.gpsimd.collective_compute("AllToAll", ALU.bypass, replica_groups=RG,
                                         ins=[cq_in.opt()], outs=[cq_out.opt()])
            q2 = bigp.tile([P, L], dt.bfloat16, tag="q_sb")
            for p in range(NC):
                nc.sync.dma_start(q2[:, p * LC:(p + 1) * LC], cq_out[p])

            o2_h0 = opool.tile([65, L], dt.bfloat16, tag="osb")
            o2_h1 = opool.tile([65, L], dt.bfloat16, tag="osb")
            for lc in range(LCH):
                sl = slice(lc * 512, (lc + 1) * 512)
                pso0 = psB.tile([P, 512], dt.float32, tag="psb")
                pso1 = psB.tile([P, 512], dt.float32, tag="psb")
                for mt in range(MTC):
                    pqk = psA.tile([P, 1024], dt.float32, tag="psa")
                    nc.tensor.matmul(pqk[:, 0:512], k2c[0:64, mt * P:(mt + 1) * P],
                                     q2[0:64, sl], start=True, stop=True)
                    nc.tensor.matmul(pqk[:, 512:1024], k2c[64:128, mt * P:(mt + 1) * P],
                                     q2[64:128, sl], start=True, stop=True)
                    Pt = ppool.tile([P, 1024], dt.bfloat16, tag="Pt")
                    nc.scalar.activation(Pt[:], pqk[:], AF.Exp, scale=HD ** -0.5)
                    nc.tensor.matmul(pso0[:65, :], vextc[mt][:, 0:65], Pt[:, 0:512],
                                     start=(mt == 0), stop=(mt == MTC - 1))
                    nc.tensor.matmul(pso1[:65, :], vextc[mt][:, 65:130], Pt[:, 512:1024],
                                     start=(mt == 0), stop=(mt == MTC - 1))
                nc.vector.tensor_copy(o2_h0[:, sl], pso0[:65, :])
                nc.vector.tensor_copy(o2_h1[:, sl], pso1[:65, :])

            co_in = dram.tile([NC, 2, 65, LC], dt.bfloat16, tag="co_in")
            co_out = dram.tile([NC, 2, 65, LC], dt.bfloat16, tag="co_out")
            for p in range(NC):
                nc.sync.dma_start(co_in[p, 0], o2_h0[:, p * LC:(p + 1) * LC])
                nc.sync.dma_start(co_in[p, 1], o2_h1[:, p * LC:(p + 1) * LC])
            nc.gpsimd.collective_compute("AllToAll", ALU.bypass, replica_groups=RG,
                                         ins=[co_in.opt()], outs=[co_out.opt()])
            onorm2 = recv_o(co_out, "c")
            for j in range(DCH):
                wt = wpool.tile([P, DCH, P], dt.bfloat16, tag="w8")
                nc.sync.dma_start(wt[:], wco.ap()[i, j])
                pp = psB.tile([P, 512], dt.float32, tag="psb")
                for o in range(DCH):
                    nc.tensor.matmul(pp[:, :LC], wt[:, o, :], onorm2[:, o, :],
                                     start=(o == 0), stop=(o == DCH - 1))
                tmp = stg.tile([P, LC], dt.float32, tag="resid")
                nc.vector.tensor_scalar(tmp[:], pp[:, :LC], vap(i, 8, j), None, ALU.add)
                nc.vector.tensor_tensor(x_sb[:, j, :], x_sb[:, j, :], tmp[:], ALU.add)

            # ===== MLP =====
            hx3 = spool.tile([P, DCH, LC], dt.bfloat16, tag="hx")
            emit_ln(i, (5, 4), hx3)
            g_sb = cpool.tile([P, FCH, LC], dt.bfloat16, tag="g_sb")
            for j in range(FCH):
                wt = wpool.tile([P, DCH, P], dt.bfloat16, tag="w8")
                nc.sync.dma_start(wt[:], wf1.ap()[i, j])
                pp = psB.tile([P, 512], dt.float32, tag="psb")
                for o in range(DCH):
                    nc.tensor.matmul(pp[:, :LC], wt[:, o, :], hx3[:, o, :],
                                     start=(o == 0), stop=(o == DCH - 1))
                nc.scalar.activation(g_sb[:, j, :], pp[:, :LC], AF.Gelu,
                                     bias=f1b_sb[:, i * FCH + j:i * FCH + j + 1])
            for j in range(DCH):
                pp = psB.tile([P, 512], dt.float32, tag="psb")
                for og in range(4):
                    wt = wpool.tile([P, DCH, P], dt.bfloat16, tag="w8")
                    nc.sync.dma_start(wt[:], wf2.ap()[i, j, og])
                    for o2 in range(DCH):
                        nc.tensor.matmul(pp[:, :LC], wt[:, o2, :], g_sb[:, og * DCH + o2, :],
                                         start=(og == 0 and o2 == 0),
                                         stop=(og == 3 and o2 == DCH - 1))
                tmp = stg.tile([P, LC], dt.float32, tag="resid")
                nc.vector.tensor_scalar(tmp[:], pp[:, :LC], vap(i, 6, j), vap(i, 7, j),
                                        ALU.mult, ALU.add)
                nc.vector.tensor_tensor(x_sb[:, j, :], x_sb[:, j, :], tmp[:], ALU.add)

        nc.sync.dma_start(out_t.ap().rearrange("p (o t) -> p o t", t=LC), x_sb[:])

    nc.compile()
    return nc


def _host_prep(inputs):
    """Build per-core in_maps from full inputs."""
    f32 = np.float32
    x = np.asarray(inputs["x"], f32)[0]           # [L, D]
    te = np.asarray(inputs["timestep_emb"], f32)  # [1, D]
    ctx = np.asarray(inputs["context_emb"], f32)[0]
    rope = np.asarray(inputs["rope_emb"], f32)    # [L, HD]
    cos, sin = np.cos(rope), np.sin(rope)

    def sbufize(a2d):  # [D, T] -> [128, DCH*T] channel-major sbuf layout
        Dd, T = a2d.shape
        return np.ascontiguousarray(
            a2d.reshape(Dd // P, P, T).transpose(1, 0, 2).reshape(P, (Dd // P) * T))

    def lhst5(WT, jn):  # WT [D, DOUT] -> [jn, P, DCH, P] tiles of W^T
        Dd, DO = WT.shape
        a = WT.reshape(DCH, P, jn, P)         # [o, p, j, f]
        return np.ascontiguousarray(a.transpose(2, 1, 0, 3)).astype(BF)  # [j, p, o, f]

    cos2 = np.tile(cos.T, (2, 1)).astype(BF)                      # [128, L]
    ssg = np.concatenate([-sin.T[:32], sin.T[32:]], 0)
    ss2 = np.tile(ssg, (2, 1)).astype(BF)
    sig = (np.arange(P) + 32) % 64 + 64 * (np.arange(P) // 64)
    permT = np.zeros((P, P), f32)
    permT[sig, np.arange(P)] = 1.0

    vecs = np.zeros((NB, NV, P, DCH), f32)
    f1bv = np.zeros((NB, P, FCH), f32)
    wqk = np.zeros((NB, 16, P, DCH, P), BF)
    wvv = np.zeros((NB, 2, P, DCH, 512), BF)
    wso = np.zeros((NB, DCH, P, DCH, P), BF)
    wcq = np.zeros((NB, DCH, P, DCH, P), BF)
    wco = np.zeros((NB, DCH, P, DCH, P), BF)
    wf1 = np.zeros((NB, FCH, P, DCH, P), BF)
    wf2 = np.zeros((NB, DCH, 4, P, DCH, P), BF)
    wck_c = [np.zeros((NB, P, DCH, P), BF) for _ in range(NC)]
    wcv_c = [np.zeros((NB, P, DCH, P), BF) for _ in range(NC)]

    for i in range(NB):
        mods = (te @ np.asarray(inputs["adaW"], f32)[i].T
                + np.asarray(inputs["adab"], f32)[i])[0]
        sh_msa, sc_msa, g_msa, sh_mlp, sc_mlp, g_mlp = np.split(mods, 6)
        sob = np.asarray(inputs["sob"], f32)[i]
        cob = np.asarray(inputs["cob"], f32)[i]
        f2b = np.asarray(inputs["f2b"], f32)[i]
        vlist = [sh_msa, 1.0 + sc_msa, g_msa, g_msa * sob,
                 sh_mlp, 1.0 + sc_mlp, g_mlp, g_mlp * f2b, cob]
        for v, arr in enumerate(vlist):
            vecs[i, v] = arr.reshape(DCH, P).T
        f1bv[i] = np.asarray(inputs["f1b"], f32)[i].reshape(FCH, P).T

        sq, sk, sv = (np.asarray(inputs[k], f32)[i] for k in ("sqW", "skW", "svW"))
        wqk[i, :8] = lhst5(sq.T, 8)
        wqk[i, 8:] = lhst5(sk.T, 8)
        svT = sv.T  # [din, dout]
        wvv[i] = np.ascontiguousarray(
            svT.reshape(DCH, P, 2, 512).transpose(2, 1, 0, 3)).astype(BF)
        wso[i] = lhst5(np.asarray(inputs["soW"], f32)[i].T, DCH)
        wcq[i] = lhst5(np.asarray(inputs["cqW"], f32)[i].T, DCH)
        wco[i] = lhst5(np.asarray(inputs["coW"], f32)[i].T, DCH)
        wf1[i] = lhst5(np.asarray(inputs["f1W"], f32)[i].T, FCH)
        f2T = np.asarray(inputs["f2W"], f32)[i].T  # [DFF, D]
        wf2[i] = np.ascontiguousarray(
            f2T.reshape(4, DCH, P, DCH, P).transpose(3, 0, 2, 1, 4)).astype(BF)
        ckT = np.asarray(inputs["ckW"], f32)[i].T
        cvT = np.asarray(inputs["cvW"], f32)[i].T
        for c in range(NC):
            sl = slice(c * P, (c + 1) * P)
            wck_c[c][i] = np.ascontiguousarray(
                ckT[:, sl].reshape(DCH, P, P).transpose(1, 0, 2)).astype(BF)
            wcv_c[c][i] = np.ascontiguousarray(
                cvT[:, sl].reshape(DCH, P, P).transpose(1, 0, 2)).astype(BF)

    ctx_t = sbufize(ctx.T).astype(BF)
    shared = dict(ctx_t=ctx_t, cos2=np.ascontiguousarray(cos2),
                  ss2=np.ascontiguousarray(ss2), permT=permT.astype(BF),
                  vecs=np.ascontiguousarray(
                      vecs.transpose(2, 0, 1, 3).reshape(P, NB * NV * DCH)),
                  f1bv=np.ascontiguousarray(
                      f1bv.transpose(1, 0, 2).reshape(P, NB * FCH)),
                  wqk=wqk, wv=wvv, wso=wso, wcq=wcq, wco=wco, wf1=wf1, wf2=wf2)
    in_maps = []
    for c in range(NC):
        m = dict(shared)
        m["x_t"] = sbufize(np.ascontiguousarray(x.T[:, c * LC:(c + 1) * LC]))
        m["wck"] = wck_c[c]
        m["wcv"] = wcv_c[c]
        in_maps.append(m)
    return in_maps


_last = {}


def kernel(**inputs):
    import os
    if "nc" not in _cache:
        _cache["nc"] = _build()
    nc = _cache["nc"]
    in_maps = _host_prep(inputs)
    trace = bool(os.environ.get("KERNEL_TRACE"))
    res = run_bass_kernel_spmd(nc, in_maps, core_ids=list(range(NC)), trace=trace)
    _last["res"] = res
    outs = []
    for c in range(NC):
        o = res.results[c]["out_t"]  # [128, DCH*LC]
        outs.append(o.reshape(P, DCH, LC).transpose(1, 0, 2).reshape(D, LC))
    xT = np.concatenate(outs, axis=1)  # [D, L]
    return np.ascontiguousarray(xT.T)[None].astype(np.float32)

